# revision 14
# baseline (speedup 1.0000x reference)
# MoE grouped-GEMM kernel for Trainium2 (Bass/Tile), 8 NeuronCores SPMD.
#
# Problem: x [65536, 1024] fp32, 64 experts each owning a contiguous group of
# 1024 tokens. Per expert: h = relu(x_g @ W1^T) (1024->64), y_g = h @ W2^T
# (64->1024).
#
# Sharding: expert-parallel == token-parallel (tokens pre-sorted by expert,
# equal groups). Core c handles experts 8c..8c+7 and their 8192 tokens. No
# collectives; host slices inputs and concatenates outputs.
#
# Per-core kernel (~68 MB HBM traffic/core, memory-bound):
#  - Experts are processed in PAIRS to fill the 128-wide PE array with fp32
#    matmuls (which only use 64 of 128 lanes per expert here):
#      FC1: expert A -> PSUM partitions 0:64, expert B -> 64:128 (column
#           tiling; concurrent on distinct col groups).
#      FC2: lhsT h_A on partitions 0:64, h_B on 64:128 (row tiling;
#           concurrent on distinct row groups). w2 is staged pair-stacked
#           [128, O] so rhs base partitions match lhsT.
#  - Loads stream on the two HWDGE rings (sync + scalar, alternating);
#    stores go out via nc.gpsimd (SWDGE) so a store waiting on compute
#    never blocks loads. FC2 PSUM is 4x single-bank tiles so DVE copies
#    pipeline against matmuls instead of double-buffer lockstep.
import numpy as np

import concourse.bacc as bacc
import concourse.bass as bass
import concourse.mybir as mybir
import concourse.tile as tile
from concourse.bass import ds, ts
from concourse.bass_utils import run_bass_kernel_spmd

E = 64          # experts
H = 64          # expert hidden
D = 1024        # d_in
O = 1024        # d_out
T = 65536       # total tokens
N_CORES = 8
E_PER = E // N_CORES        # 8 experts per core
TPE = T // E                # 1024 tokens per expert
T_PER = TPE * E_PER         # 8192 tokens per core
DC = D // 128               # 8 contraction chunks of 128
MC = TPE // 128             # 8 token chunks of 128 per expert
FP32 = mybir.dt.float32

_NC_CACHE = {}


def build_nc():
    nc = bacc.Bacc("TRN2", target_bir_lowering=False, debug=False,
                   num_devices=N_CORES)

    xs = nc.declare_dram_parameter("xs", [E_PER, 128, DC, TPE], FP32,
                                   isOutput=False)
    w1s = nc.declare_dram_parameter("w1s", [E_PER, 128, DC * H], FP32,
                                    isOutput=False)
    # w2 pair-stacked: [pair, 128, O]; rows 0:64 = W2^T of expert 2q,
    # rows 64:128 = W2^T of expert 2q+1.
    w2s = nc.declare_dram_parameter("w2s", [E_PER // 2, 128, O], FP32,
                                    isOutput=False)
    y = nc.declare_dram_parameter("y", [T_PER, O], FP32, isOutput=True)

    with tile.TileContext(nc) as tc:
        with (
            tc.tile_pool(name="w1pool", bufs=4) as w1pool,
            tc.tile_pool(name="w2pool", bufs=2) as w2pool,
            tc.tile_pool(name="xpool", bufs=14) as xpool,
            tc.tile_pool(name="hpool", bufs=2) as hpool,
            tc.tile_pool(name="ypool", bufs=5) as ypool,
            tc.tile_pool(name="phpool", bufs=2, space=bass.MemorySpace.PSUM) as phpool,
            tc.tile_pool(name="pypool", bufs=4, space=bass.MemorySpace.PSUM) as pypool,
        ):
            x_ts = []      # per expert: 4 dc-quarter tiles [128, 2, TPE]
            w1_ts = []     # per pair [128, 2, DC*H]
            w2_ts = []     # per pair [128, O]
            ring = [0]     # alternate x loads across the two HWDGE rings

            def load_expert(le):
                tiles = []
                for c in range(DC // 2):
                    xt = xpool.tile([128, 2, TPE], FP32, tag="x")
                    nc.sync.dma_start(out=xt[:],
                                      in_=xs[le, :, 2 * c:2 * c + 2, :])
                    tiles.append(xt)
                x_ts.append(tiles)
                if le % 2 == 0:
                    q = le // 2
                    w1_t = w1pool.tile([128, 2, DC * H], FP32, tag="w1")
                    nc.sync.dma_start(
                        out=w1_t[:],
                        in_=w1s[2 * q:2 * q + 2].rearrange("e p f -> p e f"))
                    w1_ts.append(w1_t)
                    w2_t = w2pool.tile([128, O], FP32, tag="w2")
                    nc.sync.dma_start(out=w2_t[:], in_=w2s[q])
                    w2_ts.append(w2_t)

            load_expert(0)
            load_expert(1)
            load_expert(2)

            def fc1_units(q, ph):
                # 16 units; each emits the A/B column-tiled MM pair for
                # one (dc, tb). dc-outer so x quarter-tiles release early.
                eA, eB = 2 * q, 2 * q + 1
                xA, xB = x_ts[eA], x_ts[eB]
                w1_t = w1_ts[q]
                units = []
                for dc in range(DC):
                    for tb in range(TPE // 512):
                        def u(dc=dc, tb=tb):
                            nc.tensor.matmul(
                                ph[0:H, ts(tb, 512)],
                                w1_t[:, 0, ds(dc * H, H)],
                                xA[dc // 2][:, dc % 2, ts(tb, 512)],
                                start=(dc == 0), stop=(dc == DC - 1),
                                skip_group_check=True,
                            )
                            nc.tensor.matmul(
                                ph[H:128, ts(tb, 512)],
                                w1_t[:, 1, ds(dc * H, H)],
                                xB[dc // 2][:, dc % 2, ts(tb, 512)],
                                start=(dc == 0), stop=(dc == DC - 1),
                                skip_group_check=True,
                            )
                        units.append(u)
                return units

            def fc2_units(q, h_t):
                # 16 units; each emits 2 row-tiled MMs + 2 drains for one
                # (quarter, mm, side). PSUM drains split across DVE (oc 0)
                # and ACT (oc 1). Stores (SWDGE) fire per quarter.
                eA, eB = 2 * q, 2 * q + 1
                w2_t = w2_ts[q]
                units = []
                state = {}
                for quarter in range(MC // 2):
                    for mm in range(2):
                        for side in range(2):
                            def u(quarter=quarter, mm=mm, side=side):
                                if mm == 0 and side == 0:
                                    state["yA"] = ypool.tile(
                                        [128, 2, O], FP32, tag="y",
                                        name="y_tA")
                                    state["yB"] = ypool.tile(
                                        [128, 2, O], FP32, tag="y",
                                        name="y_tB")
                                yt = state["yA"] if side == 0 else state["yB"]
                                base = 0 if side == 0 else H
                                m = quarter * 2 + mm
                                for oc in range(O // 512):
                                    py = pypool.tile([128, 512], FP32,
                                                     tag="py")
                                    nc.tensor.matmul(
                                        py[:],
                                        h_t[base:base + H, ts(m, 128)],
                                        w2_t[base:base + H, ts(oc, 512)],
                                        start=True, stop=True,
                                    )
                                    if oc == 0:
                                        nc.vector.tensor_copy(
                                            yt[:, mm, ds(oc * 512, 512)],
                                            py[:])
                                    else:
                                        nc.scalar.activation(
                                            yt[:, mm, ds(oc * 512, 512)],
                                            py[:],
                                            mybir.ActivationFunctionType.Identity)
                                if mm == 1 and side == 1:
                                    for le, y_t in ((eA, state["yA"]),
                                                    (eB, state["yB"])):
                                        y_view = y[
                                            ds(le * TPE + quarter * 256, 256),
                                            :].rearrange("(m p) o -> p m o",
                                                         p=128)
                                        nc.gpsimd.dma_start(out=y_view,
                                                            in_=y_t[:])
                            units.append(u)
                return units

            for q in range(E_PER // 2):
                if 2 * q + 3 < E_PER:
                    load_expert(2 * q + 3)
                if 2 * q + 4 < E_PER:
                    load_expert(2 * q + 4)
                ph = phpool.tile([128, TPE], FP32, tag="ph")
                for u in fc1_units(q, ph):
                    u()
                # ReLU on DVE (max with 0) keeps the ACT engine free for
                # pure Identity drain copies (no table switches, no DMA
                # issue stalls in its in-order stream).
                h_t = hpool.tile([128, TPE], FP32, tag="h")
                nc.vector.tensor_scalar_max(h_t[:], ph[:], 0.0)
                for u in fc2_units(q, h_t):
                    u()

    nc.compile()
    return nc


def get_nc():
    if "nc" not in _NC_CACHE:
        _NC_CACHE["nc"] = build_nc()
    return _NC_CACHE["nc"]


def _prep_inputs(x, batched_fc1_w, batched_fc2_w):
    x = np.ascontiguousarray(np.asarray(x, dtype=np.float32))
    fc1 = np.ascontiguousarray(np.asarray(batched_fc1_w, dtype=np.float32))
    fc2 = np.ascontiguousarray(np.asarray(batched_fc2_w, dtype=np.float32))

    # xs[e, p, c, t] = x[e*TPE + t, c*128 + p]
    xs = np.ascontiguousarray(
        x.reshape(E, TPE, DC, 128).transpose(0, 3, 2, 1))
    # w1s[e, p, c*H + h] = W1[e, h, c*128 + p]
    w1s = np.ascontiguousarray(
        fc1.reshape(E, H, DC, 128).transpose(0, 3, 2, 1).reshape(E, 128, DC * H))
    # w2 pair-stacked: [E//2, 128, O]; [q, 0:64, o] = W2[2q, o, h],
    # [q, 64:128, o] = W2[2q+1, o, h]
    w2s = np.ascontiguousarray(
        fc2.transpose(0, 2, 1).reshape(E // 2, 2 * H, O))

    in_maps = []
    for c in range(N_CORES):
        sl = slice(c * E_PER, (c + 1) * E_PER)
        slp = slice(c * E_PER // 2, (c + 1) * E_PER // 2)
        in_maps.append({"xs": xs[sl], "w1s": w1s[sl], "w2s": w2s[slp]})
    return in_maps


def run(inputs, trace=False):
    """Returns (y_full, BassKernelResults)."""
    in_maps = _prep_inputs(inputs["x"], inputs["batched_fc1_w"],
                           inputs["batched_fc2_w"])
    nc = get_nc()
    res = run_bass_kernel_spmd(nc, in_maps, list(range(N_CORES)), trace=trace)
    y_full = np.concatenate([res.results[c]["y"] for c in range(N_CORES)],
                            axis=0)
    return y_full, res


def kernel(x, fwd_expert_count, batched_fc1_w, batched_fc2_w):
    y, _ = run({"x": x, "batched_fc1_w": batched_fc1_w,
                "batched_fc2_w": batched_fc2_w})
    return y


# revision 15
# speedup vs baseline: 1.1106x; 1.1106x over previous
# MoE grouped-GEMM kernel for Trainium2 (Bass/Tile), 8 NeuronCores SPMD.
#
# Problem: x [65536, 1024] fp32, 64 experts each owning a contiguous group of
# 1024 tokens. Per expert: h = relu(x_g @ W1^T) (1024->64), y_g = h @ W2^T
# (64->1024).
#
# Sharding: expert-parallel == token-parallel (tokens pre-sorted by expert,
# equal groups). Core c handles experts 8c..8c+7 and their 8192 tokens. No
# collectives; host slices inputs and concatenates outputs.
#
# Per-core kernel (~68 MB HBM traffic/core, memory-bound):
#  - Experts are processed in PAIRS to fill the 128-wide PE array with fp32
#    matmuls (which only use 64 of 128 lanes per expert here):
#      FC1: expert A -> PSUM partitions 0:64, expert B -> 64:128 (column
#           tiling; concurrent on distinct col groups).
#      FC2: lhsT h_A on partitions 0:64, h_B on 64:128 (row tiling;
#           concurrent on distinct row groups). w2 is staged pair-stacked
#           [128, O] so rhs base partitions match lhsT.
#  - Loads stream on the two HWDGE rings (sync + scalar, alternating);
#    stores go out via nc.gpsimd (SWDGE) so a store waiting on compute
#    never blocks loads. FC2 PSUM is 4x single-bank tiles so DVE copies
#    pipeline against matmuls instead of double-buffer lockstep.
import numpy as np

import concourse.bacc as bacc
import concourse.bass as bass
import concourse.mybir as mybir
import concourse.tile as tile
from concourse.bass import ds, ts
from concourse.bass_utils import run_bass_kernel_spmd

E = 64          # experts
H = 64          # expert hidden
D = 1024        # d_in
O = 1024        # d_out
T = 65536       # total tokens
N_CORES = 8
E_PER = E // N_CORES        # 8 experts per core
TPE = T // E                # 1024 tokens per expert
T_PER = TPE * E_PER         # 8192 tokens per core
DC = D // 128               # 8 contraction chunks of 128
MC = TPE // 128             # 8 token chunks of 128 per expert
FP32 = mybir.dt.float32

_NC_CACHE = {}


def build_nc():
    nc = bacc.Bacc("TRN2", target_bir_lowering=False, debug=False,
                   num_devices=N_CORES)

    xs = nc.declare_dram_parameter("xs", [E_PER, 128, DC, TPE], FP32,
                                   isOutput=False)
    w1s = nc.declare_dram_parameter("w1s", [E_PER, 128, DC * H], FP32,
                                    isOutput=False)
    # w2 pair-stacked: [pair, 128, O]; rows 0:64 = W2^T of expert 2q,
    # rows 64:128 = W2^T of expert 2q+1.
    w2s = nc.declare_dram_parameter("w2s", [E_PER // 2, 128, O], FP32,
                                    isOutput=False)
    y = nc.declare_dram_parameter("y", [T_PER, O], FP32, isOutput=True)

    with tile.TileContext(nc) as tc:
        with (
            tc.tile_pool(name="w1pool", bufs=4) as w1pool,
            tc.tile_pool(name="w2pool", bufs=2) as w2pool,
            tc.tile_pool(name="xpool", bufs=14) as xpool,
            tc.tile_pool(name="hpool", bufs=2) as hpool,
            tc.tile_pool(name="ypool", bufs=5) as ypool,
            tc.tile_pool(name="phpool", bufs=2, space=bass.MemorySpace.PSUM) as phpool,
            tc.tile_pool(name="pypool", bufs=4, space=bass.MemorySpace.PSUM) as pypool,
        ):
            x_ts = []      # per expert: 4 dc-quarter tiles [128, 2, TPE]
            w1_ts = []     # per pair [128, 2, DC*H]
            w2_ts = []     # per pair [128, O]
            ring = [0]     # alternate x loads across the two HWDGE rings

            def load_expert(le):
                tiles = []
                for c in range(DC // 2):
                    xt = xpool.tile([128, 2, TPE], FP32, tag="x")
                    nc.sync.dma_start(out=xt[:],
                                      in_=xs[le, :, 2 * c:2 * c + 2, :])
                    tiles.append(xt)
                x_ts.append(tiles)
                if le % 2 == 0:
                    q = le // 2
                    w1_t = w1pool.tile([128, 2, DC * H], FP32, tag="w1")
                    nc.sync.dma_start(
                        out=w1_t[:],
                        in_=w1s[2 * q:2 * q + 2].rearrange("e p f -> p e f"))
                    w1_ts.append(w1_t)
                    w2_t = w2pool.tile([128, O], FP32, tag="w2")
                    nc.sync.dma_start(out=w2_t[:], in_=w2s[q])
                    w2_ts.append(w2_t)

            # Prologue: the first FC1 unit's deps go first — xA[0],
            # xB[0], w1(pair0) — then the rest of pair 0 in v9 order.
            def _x_tile(le, c):
                xt = xpool.tile([128, 2, TPE], FP32, tag="x",
                                name=f"x{le}_{c}")
                nc.sync.dma_start(out=xt[:],
                                  in_=xs[le, :, 2 * c:2 * c + 2, :])
                return xt

            xa0 = [_x_tile(0, 0)]
            xb0 = [_x_tile(1, 0)]
            w1_t0 = w1pool.tile([128, 2, DC * H], FP32, tag="w1")
            nc.sync.dma_start(out=w1_t0[:],
                              in_=w1s[0:2].rearrange("e p f -> p e f"))
            w1_ts.append(w1_t0)
            for c in range(1, DC // 2):
                xa0.append(_x_tile(0, c))
            for c in range(1, DC // 2):
                xb0.append(_x_tile(1, c))
            w2_t0 = w2pool.tile([128, O], FP32, tag="w2")
            nc.sync.dma_start(out=w2_t0[:], in_=w2s[0])
            w2_ts.append(w2_t0)
            x_ts.append(xa0)
            x_ts.append(xb0)
            load_expert(2)

            def fc1_units(q, ph):
                # 16 units; each emits the A/B column-tiled MM pair for
                # one (dc, tb). dc-outer so x quarter-tiles release early.
                eA, eB = 2 * q, 2 * q + 1
                xA, xB = x_ts[eA], x_ts[eB]
                w1_t = w1_ts[q]
                units = []
                for dc in range(DC):
                    for tb in range(TPE // 512):
                        def u(dc=dc, tb=tb):
                            nc.tensor.matmul(
                                ph[0:H, ts(tb, 512)],
                                w1_t[:, 0, ds(dc * H, H)],
                                xA[dc // 2][:, dc % 2, ts(tb, 512)],
                                start=(dc == 0), stop=(dc == DC - 1),
                                skip_group_check=True,
                            )
                            nc.tensor.matmul(
                                ph[H:128, ts(tb, 512)],
                                w1_t[:, 1, ds(dc * H, H)],
                                xB[dc // 2][:, dc % 2, ts(tb, 512)],
                                start=(dc == 0), stop=(dc == DC - 1),
                                skip_group_check=True,
                            )
                        units.append(u)
                return units

            def fc2_units(q, h_t):
                # 16 units; each emits 2 row-tiled MMs + 2 drains for one
                # (quarter, mm, side). PSUM drains split across DVE (oc 0)
                # and ACT (oc 1). Stores (SWDGE) fire per quarter.
                eA, eB = 2 * q, 2 * q + 1
                w2_t = w2_ts[q]
                units = []
                state = {}
                for quarter in range(MC // 2):
                    for mm in range(2):
                        for side in range(2):
                            def u(quarter=quarter, mm=mm, side=side):
                                if mm == 0 and side == 0:
                                    state["yA"] = ypool.tile(
                                        [128, 2, O], FP32, tag="y",
                                        name="y_tA")
                                    state["yB"] = ypool.tile(
                                        [128, 2, O], FP32, tag="y",
                                        name="y_tB")
                                yt = state["yA"] if side == 0 else state["yB"]
                                base = 0 if side == 0 else H
                                m = quarter * 2 + mm
                                for oc in range(O // 512):
                                    py = pypool.tile([128, 512], FP32,
                                                     tag="py")
                                    nc.tensor.matmul(
                                        py[:],
                                        h_t[base:base + H, ts(m, 128)],
                                        w2_t[base:base + H, ts(oc, 512)],
                                        start=True, stop=True,
                                    )
                                    if oc == 0:
                                        nc.vector.tensor_copy(
                                            yt[:, mm, ds(oc * 512, 512)],
                                            py[:])
                                    else:
                                        nc.scalar.activation(
                                            yt[:, mm, ds(oc * 512, 512)],
                                            py[:],
                                            mybir.ActivationFunctionType.Identity)
                                if mm == 1 and side == 1:
                                    for le, y_t in ((eA, state["yA"]),
                                                    (eB, state["yB"])):
                                        y_view = y[
                                            ds(le * TPE + quarter * 256, 256),
                                            :].rearrange("(m p) o -> p m o",
                                                         p=128)
                                        nc.gpsimd.dma_start(out=y_view,
                                                            in_=y_t[:])
                            units.append(u)
                return units

            for q in range(E_PER // 2):
                if 2 * q + 3 < E_PER:
                    load_expert(2 * q + 3)
                if 2 * q + 4 < E_PER:
                    load_expert(2 * q + 4)
                ph = phpool.tile([128, TPE], FP32, tag="ph")
                for u in fc1_units(q, ph):
                    u()
                # ReLU on DVE (max with 0) keeps the ACT engine free for
                # pure Identity drain copies (no table switches, no DMA
                # issue stalls in its in-order stream).
                h_t = hpool.tile([128, TPE], FP32, tag="h")
                nc.vector.tensor_scalar_max(h_t[:], ph[:], 0.0)
                for u in fc2_units(q, h_t):
                    u()

    nc.compile()
    return nc


def get_nc():
    if "nc" not in _NC_CACHE:
        _NC_CACHE["nc"] = build_nc()
    return _NC_CACHE["nc"]


def _prep_inputs(x, batched_fc1_w, batched_fc2_w):
    x = np.ascontiguousarray(np.asarray(x, dtype=np.float32))
    fc1 = np.ascontiguousarray(np.asarray(batched_fc1_w, dtype=np.float32))
    fc2 = np.ascontiguousarray(np.asarray(batched_fc2_w, dtype=np.float32))

    # xs[e, p, c, t] = x[e*TPE + t, c*128 + p]
    xs = np.ascontiguousarray(
        x.reshape(E, TPE, DC, 128).transpose(0, 3, 2, 1))
    # w1s[e, p, c*H + h] = W1[e, h, c*128 + p]
    w1s = np.ascontiguousarray(
        fc1.reshape(E, H, DC, 128).transpose(0, 3, 2, 1).reshape(E, 128, DC * H))
    # w2 pair-stacked: [E//2, 128, O]; [q, 0:64, o] = W2[2q, o, h],
    # [q, 64:128, o] = W2[2q+1, o, h]
    w2s = np.ascontiguousarray(
        fc2.transpose(0, 2, 1).reshape(E // 2, 2 * H, O))

    in_maps = []
    for c in range(N_CORES):
        sl = slice(c * E_PER, (c + 1) * E_PER)
        slp = slice(c * E_PER // 2, (c + 1) * E_PER // 2)
        in_maps.append({"xs": xs[sl], "w1s": w1s[sl], "w2s": w2s[slp]})
    return in_maps


def run(inputs, trace=False):
    """Returns (y_full, BassKernelResults)."""
    in_maps = _prep_inputs(inputs["x"], inputs["batched_fc1_w"],
                           inputs["batched_fc2_w"])
    nc = get_nc()
    res = run_bass_kernel_spmd(nc, in_maps, list(range(N_CORES)), trace=trace)
    y_full = np.concatenate([res.results[c]["y"] for c in range(N_CORES)],
                            axis=0)
    return y_full, res


def kernel(x, fwd_expert_count, batched_fc1_w, batched_fc2_w):
    y, _ = run({"x": x, "batched_fc1_w": batched_fc1_w,
                "batched_fc2_w": batched_fc2_w})
    return y
